# revision 34
# baseline (speedup 1.0000x reference)
"""Trainium2 Bass kernel for nn_Decoder (GRU decoder + MLP + vocab softmax).

Sharding (8 NeuronCores):
  - GRU + 2-layer MLP: data-parallel over batch (4 examples/core).
    Local tokens are b-major (col = b*128 + t) so the global token index
    G = 512*rank + b*128 + t equals example*128 + t, matching output rows.
  - h2^T all-gathered (bf16, 2 chunked collectives) across cores.
  - Final [512,32000] vocab projection + softmax: column-parallel
    (4000 vocab cols/core) with AllReduce'd softmax denominators.

Compute dtypes: bf16 matmul operands, fp32 PSUM accumulation and gates,
bf16 hidden state and exp store (verified ~2.4e-3 rel err vs the fp32
reference; gate is 2e-2).

The softmax skips max-subtraction: logits for this model are O(+-2), far
inside exp's fp32 range, and exp+rowsum are fused in one ScalarE pass via
accum_out.
"""

import numpy as np

import concourse.bass as bass
import concourse.tile as tile
from concourse import bacc, mybir
from concourse.bass import ds, ts
from concourse.bass_utils import run_bass_kernel_spmd
from concourse.masks import make_identity

P = 128
NCORES = 8
B, T, E, H, V = 32, 128, 256, 512, 32000
BL = B // NCORES            # 4 examples per core
NTOK = BL * T               # 512 local tokens
G = B * T                   # 4096 global tokens
VS = V // NCORES            # 4000 vocab cols per core
KO = H // P                 # 4 hidden chunks
MO3 = 3 * H // P            # 12 gate chunks (z:0-3, r:4-7, h:8-11)
SO = (E + H) // P           # 6 input chunks
NJ = 8                      # vocab sub-chunks per token tile (8 x 500)
VC = VS // NJ               # 500
NQ = 2                      # gather chunks
ROUND_SIZES = [6, 6, 6, 4, 4, 4, 2]  # token-tiles per softmax all-reduce round
ROUNDS = len(ROUND_SIZES)
TPR = max(ROUND_SIZES)

f32 = mybir.dt.float32
bf16 = mybir.dt.bfloat16
fp8 = mybir.dt.float8e4

R_FP8 = False

TRACE = False
TRACE_KWARGS = {}
LAST_RESULT = None

RG = [list(range(NCORES))]


def _build(has_b3: bool, has_gb: bool, debug: str | None = None):
    nc = bacc.Bacc("TRN2", target_bir_lowering=False, debug=False,
                   num_devices=NCORES)

    enc_ext = nc.dram_tensor("encoder_input", [BL, T, E], f32, kind="ExternalInput").ap()
    dec_ext = nc.dram_tensor("decoder_input", [BL, H], f32, kind="ExternalInput").ap()
    gk_ext = nc.dram_tensor("gru_kernel", [E + H, 3 * H], f32, kind="ExternalInput").ap()
    gr_ext = nc.dram_tensor("gru_rec_kernel", [H, 3 * H], f32, kind="ExternalInput").ap()
    gb_ext = nc.dram_tensor("gru_bias", [2, 3 * H], f32, kind="ExternalInput").ap()
    w1_ext = nc.dram_tensor("w1", [H, H], f32, kind="ExternalInput").ap()
    b1_ext = nc.dram_tensor("b1", [H], f32, kind="ExternalInput").ap()
    w2_ext = nc.dram_tensor("w2", [H, H], f32, kind="ExternalInput").ap()
    b2_ext = nc.dram_tensor("b2", [H], f32, kind="ExternalInput").ap()
    w3_ext = nc.dram_tensor("w3", [H, VS], f32, kind="ExternalInput").ap()
    b3_ext = nc.dram_tensor("b3", [VS], f32, kind="ExternalInput").ap()

    out_ext = nc.dram_tensor("out", [G, VS], f32, kind="ExternalOutput").ap()
    dbg_ext = None
    if debug == "xproj":
        dbg_ext = nc.dram_tensor("dbg", [P, MO3, NTOK], f32, kind="ExternalOutput").ap()
    elif debug == "hseq":
        dbg_ext = nc.dram_tensor("dbg", [P, KO, NTOK], f32, kind="ExternalOutput").ap()
    elif debug == "h2g":
        dbg_ext = nc.dram_tensor("dbg", [P, KO, G], f32, kind="ExternalOutput").ap()

    with tile.TileContext(nc) as tc:
        with tc.tile_pool(name="dram", bufs=1, space="DRAM") as dram_pool:
            h2_bounce = [dram_pool.tile([H, NTOK // NQ], bf16, name=f"h2b_{q}")
                         for q in range(NQ)]
            h2_gath = [dram_pool.tile([NCORES * H, NTOK // NQ], bf16,
                                      addr_space="Shared",
                                      name=f"h2g_{q}") for q in range(NQ)]
            sums_in = [dram_pool.tile([P * ROUND_SIZES[r]], f32,
                                      name=f"sums_in_{r}")
                       for r in range(ROUNDS)]
            sums_out = [dram_pool.tile([P * ROUND_SIZES[r]], f32,
                                       addr_space="Shared",
                                       name=f"sums_out_{r}")
                        for r in range(ROUNDS)]
            _build_body(nc, tc, has_b3, has_gb, debug, dbg_ext,
                        enc_ext, dec_ext, gk_ext, gr_ext, gb_ext,
                        w1_ext, b1_ext, w2_ext, b2_ext, w3_ext, b3_ext,
                        out_ext, h2_bounce, h2_gath, sums_in, sums_out)
    nc.finalize()
    return nc


def _build_body(nc, tc, has_b3, has_gb, debug, dbg_ext,
                enc_ext, dec_ext, gk_ext, gr_ext, gb_ext,
                w1_ext, b1_ext, w2_ext, b2_ext, w3_ext, b3_ext,
                out_ext, h2_bounce, h2_gath, sums_in, sums_out):
    from contextlib import ExitStack

    Ident = mybir.ActivationFunctionType.Identity
    Sig = mybir.ActivationFunctionType.Sigmoid
    Relu = mybir.ActivationFunctionType.Relu
    Exp = mybir.ActivationFunctionType.Exp

    persist = ExitStack()
    wpool = persist.enter_context(tc.tile_pool(name="wpool", bufs=1))
    w3b = wpool.tile([P, KO, VS], bf16)
    b3bc = wpool.tile([P, VS], f32, name="b3bc") if has_b3 else None

    gru_stack = ExitStack()
    gpool = gru_stack.enter_context(tc.tile_pool(name="gpool", bufs=1))
    w1b = gpool.tile([P, KO, H], bf16)
    w2b = gpool.tile([P, KO, H], bf16)
    b1T = gpool.tile([P, KO], f32)
    b2T = gpool.tile([P, KO], f32)
    h2T = gpool.tile([P, KO, NTOK], bf16)
    gt_pool = gru_stack.enter_context(tc.tile_pool(name="gt", bufs=3))
    psum_pro = gru_stack.enter_context(tc.tile_pool(name="ps_pro", bufs=2, space="PSUM"))
    psum_rec = gru_stack.enter_context(tc.tile_pool(name="ps_rec", bufs=2, space="PSUM"))

    Rdt = fp8 if R_FP8 else bf16
    Rb = gpool.tile([P, KO, 3 * H], Rdt)
    Wkb = gpool.tile([P, SO, 3 * H], bf16)
    seqT = gpool.tile([P, SO, NTOK], bf16)
    xprojT = gpool.tile([P, MO3, NTOK], bf16)
    hseqT = gpool.tile([P, KO, NTOK], bf16)
    h1T = gpool.tile([P, KO, NTOK], bf16)

    # ---- input loads: fp32 DRAM -> SBUF, chunked, cast on DVE ----
    wtmp_pool = gru_stack.enter_context(tc.tile_pool(name="wtmp", bufs=3))

    def load_cast(dst3, src_ext, nck, width, tag):
        src_c = src_ext.rearrange("(k p) m -> k p m", p=P)
        for k in range(nck):
            tmp = wtmp_pool.tile([P, width], f32, tag=tag, name=f"{tag}_{k}")
            nc.sync.dma_start(out=tmp[:], in_=src_c[k])
            nc.vector.tensor_copy(out=dst3[:, k], in_=tmp[:])

    # encoder: natural load (contiguous rows), cast, PE-transpose into seqT
    seqT4 = seqT.rearrange("p so (b t) -> p so b t", b=BL)
    ident = gpool.tile([P, P], bf16)
    make_identity(nc, ident)
    enc_nat = gpool.tile([P, BL, E], f32)
    nc.sync.dma_start(out=enc_nat[:], in_=enc_ext.rearrange("b t c -> t b c"))
    enc_natb = gpool.tile([P, BL, E], bf16)
    nc.vector.tensor_copy(out=enc_natb[:], in_=enc_nat[:])
    for b in range(BL):
        for co in range(E // P):
            pst = psum_pro.tile([P, P], bf16, tag="pro", name=f"tp_{b}_{co}")
            nc.tensor.transpose(pst[:], enc_natb[:, b, ts(co, P)], ident)
            nc.vector.tensor_copy(out=seqT4[:, co, b, :], in_=pst[:])
    # decoder -> bf16, broadcast over t into seqT chunks 2-5
    decT = gpool.tile([P, KO, BL], f32)
    for b in range(BL):
        nc.sync.dma_start(out=decT[:, :, b],
                          in_=dec_ext[b].rearrange("(ko p) -> p ko", p=P))
    decTb = gpool.tile([P, KO, BL], bf16)
    nc.vector.tensor_copy(out=decTb[:], in_=decT[:])
    nc.vector.tensor_copy(out=seqT4[:, E // P:SO],
                          in_=decTb[:, :, :, None].to_broadcast((P, KO, BL, T)))

    load_cast(Wkb, gk_ext, SO, 3 * H, "wtmp")
    load_cast(Rb, gr_ext, KO, 3 * H, "wtmp")
    if has_b3:
        b3_brd = bass.AP(tensor=b3_ext.tensor, offset=b3_ext.offset,
                         ap=[[0, P]] + list(b3_ext.ap))
        nc.sync.dma_start(out=b3bc[:], in_=b3_brd)

    # gru biases (generic path; skipped when zero)
    if has_gb:
        gbT = gpool.tile([P, MO3, 2], f32)
        for i in range(2):
            nc.sync.dma_start(out=gbT[:, :, i],
                              in_=gb_ext[i].rearrange("(mo p) -> p mo", p=P))
        xbias = gpool.tile([P, MO3], f32)
        nc.vector.tensor_copy(out=xbias[:], in_=gbT[:, :, 0])
        nc.vector.tensor_add(out=xbias[:, 0:8], in0=xbias[:, 0:8], in1=gbT[:, 0:8, 1])
        brecH = gpool.tile([P, KO, BL], f32)
        nc.vector.tensor_copy(out=brecH[:],
                              in_=gbT[:, 8:12, 1:2].to_broadcast((P, KO, BL)))

    # ---- x_proj^T = Wk^T @ seq^T (+bias), chunked by timestep range ----
    # chunk 0 is emitted here (gates the first GRU steps); chunks 1..3 are
    # emitted after the GRU loop so the scheduler back-fills them into the
    # GRU's PE gaps instead of delaying the scan start.
    XC = 4
    XS = T // XC  # 32 steps per chunk
    seq_bt = seqT.rearrange("p so (b t) -> p so b t", b=BL)
    xp_bt = xprojT.rearrange("p m (b t) -> p m b t", b=BL)

    def emit_xproj_chunk(c):
        for m in range(MO3):
            ps = psum_pro.tile([P, BL * XS], f32, tag="pro", name=f"xp_{c}_{m}")
            for k in range(SO):
                nc.tensor.matmul(ps[:], lhsT=Wkb[:, k, ts(m, P)],
                                 rhs=seq_bt[:, k, :, ds(XS * c, XS)],
                                 start=(k == 0), stop=(k == SO - 1))
            dst = xp_bt[:, m, :, ds(XS * c, XS)]
            if has_gb:
                nc.scalar.activation(out=dst, in_=ps[:], func=Ident,
                                     bias=xbias[:, m:m + 1])
            else:
                nc.scalar.copy(out=dst, in_=ps[:])

    emit_xproj_chunk(0)

    if debug == "xproj":
        nc.sync.dma_start(out=dbg_ext, in_=xprojT[:])

    # ---- GRU scan (t-major local tokens) ----
    xp4 = xprojT.rearrange("p m (b t) -> p m b t", b=BL)
    hs4 = hseqT.rearrange("p ko (b t) -> p ko b t", b=BL)

    # t = 0 (h == 0): z,r = sig(xz), hh = relu(xh [+ r*brecH]), h = (1-z)*hh
    zr0 = gt_pool.tile([P, 8, BL], f32, tag="zr")
    nc.scalar.activation(out=zr0[:], in_=xp4[:, 0:8, :, 0], func=Sig)
    hh0 = gt_pool.tile([P, KO, BL], f32, tag="hh")
    if has_gb:
        nc.vector.tensor_mul(out=hh0[:], in0=zr0[:, 4:8], in1=brecH[:])
        nc.vector.tensor_add(out=hh0[:], in0=hh0[:], in1=xp4[:, 8:12, :, 0])
        nc.vector.tensor_scalar_max(hh0[:], hh0[:], 0.0)
    else:
        nc.vector.tensor_scalar_max(hh0[:], xp4[:, 8:12, :, 0], 0.0)
    d0 = gt_pool.tile([P, KO, BL], f32, tag="d")
    nc.vector.tensor_mul(out=d0[:], in0=zr0[:, 0:4], in1=hh0[:])
    nc.vector.tensor_sub(out=hs4[:, :, :, 0], in0=hh0[:], in1=d0[:])

    for t in range(1, T):
        if t % XS == XS - 8 and t // XS + 1 < XC:
            emit_xproj_chunk(t // XS + 1)
        r_ps = psum_rec.tile([P, KO * BL], f32, tag="r_ps", name=f"rp_{t}")
        h_ps = psum_rec.tile([P, KO * BL], f32, tag="h_ps", name=f"hp_{t}")
        z_ps = psum_rec.tile([P, KO * BL], f32, tag="z_ps", name=f"zp_{t}")
        nc.tensor.matmul(r_ps[:], lhsT=ident, rhs=xp4[:, 4:8, :, t],
                         start=True, stop=False)
        for m in range(4):
            for ko in range(KO):
                nc.tensor.matmul(r_ps[:, ds(BL * m, BL)],
                                 lhsT=Rb[:, ko, ts(4 + m, P)],
                                 rhs=hs4[:, ko, :, t - 1],
                                 start=False, stop=(ko == KO - 1) and (m == 3))
        for m in range(4):
            for ko in range(KO):
                nc.tensor.matmul(h_ps[:, ds(BL * m, BL)],
                                 lhsT=Rb[:, ko, ts(8 + m, P)],
                                 rhs=hs4[:, ko, :, t - 1],
                                 start=(ko == 0), stop=(ko == KO - 1))
        nc.tensor.matmul(z_ps[:], lhsT=ident, rhs=xp4[:, 0:4, :, t],
                         start=True, stop=False)
        for m in range(4):
            for ko in range(KO):
                nc.tensor.matmul(z_ps[:, ds(BL * m, BL)],
                                 lhsT=Rb[:, ko, ts(m, P)],
                                 rhs=hs4[:, ko, :, t - 1],
                                 start=False, stop=(ko == KO - 1) and (m == 3))
        rr = gt_pool.tile([P, KO, BL], f32, tag="rr", name=f"rr_{t}")
        nc.scalar.activation(out=rr[:],
                             in_=r_ps.rearrange("p (m b) -> p m b", b=BL), func=Sig)
        hh = gt_pool.tile([P, KO, BL], f32, tag="hh", name=f"hh_{t}")
        hp4 = h_ps.rearrange("p (m b) -> p m b", b=BL)
        if has_gb:
            nc.vector.tensor_add(out=hh[:], in0=hp4, in1=brecH[:])
            nc.vector.tensor_mul(out=hh[:], in0=rr[:], in1=hh[:])
        else:
            nc.vector.tensor_mul(out=hh[:], in0=rr[:], in1=hp4)
        nc.vector.tensor_add(out=hh[:], in0=hh[:], in1=xp4[:, 8:12, :, t])
        nc.vector.tensor_scalar_max(hh[:], hh[:], 0.0)
        dd = gt_pool.tile([P, KO, BL], f32, tag="d", name=f"d_{t}")
        nc.vector.tensor_sub(out=dd[:], in0=hs4[:, :, :, t - 1], in1=hh[:])
        zz = gt_pool.tile([P, KO, BL], f32, tag="zz", name=f"zz_{t}")
        nc.scalar.activation(out=zz[:],
                             in_=z_ps.rearrange("p (m b) -> p m b", b=BL), func=Sig)
        nc.vector.tensor_mul(out=dd[:], in0=zz[:], in1=dd[:])
        nc.vector.tensor_add(out=hs4[:, :, :, t], in0=hh[:], in1=dd[:])

    if debug == "hseq":
        dbgf = gpool.tile([P, KO, NTOK], f32)
        nc.vector.tensor_copy(out=dbgf[:], in_=hseqT[:])
        nc.sync.dma_start(out=dbg_ext, in_=dbgf[:])

    # deferred weight loads (DMAs overlap the GRU; w3 casts happen during
    # the gather window so they don't interrupt GRU gate chains)
    load_cast(w1b, w1_ext, KO, H, "wtmp")
    load_cast(w2b, w2_ext, KO, H, "wtmp")
    nc.sync.dma_start(out=b1T[:], in_=b1_ext.rearrange("(mo p) -> p mo", p=P))
    nc.sync.dma_start(out=b2T[:], in_=b2_ext.rearrange("(mo p) -> p mo", p=P))
    w3_c = w3_ext.rearrange("(k p) m -> k p m", p=P)
    for k in range(KO):
        tmp = wtmp_pool.tile([P, VS], f32, tag="w3tmp", name=f"w3tmp_{k}")
        nc.sync.dma_start(out=tmp[:], in_=w3_c[k])
        nc.vector.tensor_copy(out=w3b[:, k, 0:VS // 2], in_=tmp[:, 0:VS // 2])
        nc.vector.tensor_copy(out=w3b[:, k, VS // 2:], in_=tmp[:, VS // 2:])

    # ---- MLP (both layers chunked per gather half) ----
    HT = NTOK // NQ
    for q in range(NQ):
        for m in range(KO):
            ps = psum_pro.tile([P, HT], f32, tag="pro", name=f"m1_{q}_{m}")
            for k in range(KO):
                nc.tensor.matmul(ps[:], lhsT=w1b[:, k, ts(m, P)],
                                 rhs=hseqT[:, k, ds(HT * q, HT)],
                                 start=(k == 0), stop=(k == KO - 1))
            nc.scalar.activation(out=h1T[:, m, ds(HT * q, HT)], in_=ps[:],
                                 func=Relu, bias=b1T[:, m:m + 1])
        for m in range(KO):
            ps = psum_pro.tile([P, HT], f32, tag="pro", name=f"m2_{q}_{m}")
            for k in range(KO):
                nc.tensor.matmul(ps[:], lhsT=w2b[:, k, ts(m, P)],
                                 rhs=h1T[:, k, ds(HT * q, HT)],
                                 start=(k == 0), stop=(k == KO - 1))
            nc.scalar.activation(out=h2T[:, m, ds(HT * q, HT)], in_=ps[:],
                                 func=Relu, bias=b2T[:, m:m + 1])
        nc.gpsimd.dma_start(out=h2_bounce[q].rearrange("(ko p) t -> p ko t", p=P),
                            in_=h2T[:, :, ds(HT * q, HT)])
        nc.gpsimd.collective_compute(
            "AllGather", mybir.AluOpType.bypass,
            ins=[h2_bounce[q].opt()], outs=[h2_gath[q].opt()],
            replica_groups=RG,
        )

    gru_stack.close()

    voc_stack = ExitStack()
    vpool = voc_stack.enter_context(tc.tile_pool(name="vpool", bufs=1))
    exp_pool = voc_stack.enter_context(tc.tile_pool(name="exp", bufs=12))
    out_pool = voc_stack.enter_context(tc.tile_pool(name="outp", bufs=2))
    sc_pool = voc_stack.enter_context(tc.tile_pool(name="scp", bufs=3))
    psum_voc = voc_stack.enter_context(tc.tile_pool(name="ps_voc", bufs=2, space="PSUM"))

    h2gT = vpool.tile([P, KO, G], bf16)
    h2g_q = h2gT.rearrange("p ko (r q t) -> p ko r q t", r=NCORES, q=NQ)
    for q in range(NQ):
        src = h2_gath[q].rearrange("(r ko p) t -> ko p r t", p=P, ko=KO)
        for ko in range(KO):
            nc.scalar.dma_start(out=h2g_q[:, ko, :, q, :], in_=src[ko])

    if debug == "h2g":
        dbgf = vpool.tile([P, KO, G], f32)
        nc.vector.tensor_copy(out=dbgf[:], in_=h2gT[:])
        nc.sync.dma_start(out=dbg_ext, in_=dbgf[:])

    # vocab tiles ordered quarter-major so the first tiles only need AG q=0
    tile_order = [h * (2 * NCORES) + i for h in range(NQ)
                  for i in range(2 * NCORES)]
    # tile gt covers G rows [128*gt, 128*gt+128): rank gt//4, gather half
    # (gt%4)//2 since each rank contributes 512 tokens = 4 tiles = 2 halves.
    proc = []
    it = iter(tile_order)
    for sz in ROUND_SIZES:
        proc.append([next(it) for _ in range(sz)])

    pending = None  # (exps, proc_list, rcp) of the previous round, scaled late

    def emit_scales(pend):
        exps_p, proc_p, rcp_p, rnd_p = pend
        for i, gt in enumerate(proc_p):
            ob = out_pool.tile([P, NJ, VC], f32, tag="ob", name=f"ob_{gt}")
            if rnd_p >= ROUNDS - 2 and i % 2:
                nc.scalar.activation(out=ob[:], in_=exps_p[i][:],
                                     func=mybir.ActivationFunctionType.Copy,
                                     scale=rcp_p[:, i:i + 1])
            else:
                nc.vector.tensor_scalar_mul(ob[:], exps_p[i][:], rcp_p[:, i:i + 1])
            nc.sync.dma_start(out=out_ext[ds(P * gt, P), :],
                              in_=ob.rearrange("p j v -> p (j v)"))

    for rnd in range(ROUNDS):
        nr = ROUND_SIZES[rnd]
        sums = sc_pool.tile([P, TPR, 2], f32, tag="sums", name=f"sums_{rnd}")
        exps = []
        for i, gt in enumerate(proc[rnd]):
            expb = exp_pool.tile([P, NJ, VC], bf16, tag="expb", name=f"expb_{gt}")
            for half in range(2):
                pv = psum_voc.tile([P, NJ // 2, 512], f32, tag="pv",
                                   name=f"pv_{gt}_{half}")
                for ko in range(KO):
                    last = (ko == KO - 1) and not has_b3
                    for j in range(NJ // 2):
                        jj = half * (NJ // 2) + j
                        nc.tensor.matmul(pv[:, j, 0:VC],
                                         lhsT=h2gT[:, ko, ts(gt, P)],
                                         rhs=w3b[:, ko, ds(VC * jj, VC)],
                                         start=(ko == 0), stop=last)
                if has_b3:
                    b3v = b3bc[:, ds(VC * half * (NJ // 2), VC * (NJ // 2))]
                    nc.vector.tensor_add(
                        out=pv[:, :, 0:VC], in0=pv[:, :, 0:VC],
                        in1=b3v.rearrange("p (j v) -> p j v", j=NJ // 2))
                nc.scalar.activation(
                    out=expb[:, ds(half * (NJ // 2), NJ // 2), :],
                    in_=pv[:, :, 0:VC], func=Exp,
                    accum_out=sums[:, i, half:half + 1])
            exps.append(expb)
        ssum = sc_pool.tile([P, TPR], f32, tag="ssum", name=f"ssum_{rnd}")
        nc.vector.tensor_add(out=ssum[:, :nr], in0=sums[:, :nr, 0],
                             in1=sums[:, :nr, 1])
        nc.gpsimd.dma_start(out=sums_in[rnd].rearrange("(i p) -> p i", p=P),
                            in_=ssum[:, :nr])
        nc.gpsimd.collective_compute(
            "AllReduce", mybir.AluOpType.add,
            ins=[sums_in[rnd].opt()], outs=[sums_out[rnd].opt()],
            replica_groups=RG,
        )
        if pending is not None:
            emit_scales(pending)
        rcp = sc_pool.tile([P, TPR], f32, tag="rcp", name=f"rcp_{rnd}")
        nc.scalar.dma_start(out=rcp[:, :nr],
                            in_=sums_out[rnd].rearrange("(i p) -> p i", p=P))
        nc.vector.reciprocal(out=rcp[:, :nr], in_=rcp[:, :nr])
        pending = (exps, proc[rnd], rcp, rnd)

    emit_scales(pending)

    voc_stack.close()
    persist.close()


_BUILD_CACHE = {}


def _get_nc(has_b3: bool, has_gb: bool, debug=None):
    key = (has_b3, has_gb, debug)
    if key not in _BUILD_CACHE:
        _BUILD_CACHE[key] = _build(has_b3, has_gb, debug)
    return _BUILD_CACHE[key]


def _make_in_maps(inputs):
    arrs = {k: np.ascontiguousarray(np.asarray(v, dtype=np.float32))
            for k, v in inputs.items()}
    in_maps = []
    for c in range(NCORES):
        in_maps.append({
            "encoder_input": arrs["encoder_input"][BL * c:BL * (c + 1)],
            "decoder_input": arrs["decoder_input"][BL * c:BL * (c + 1)],
            "gru_kernel": arrs["gru_kernel"],
            "gru_rec_kernel": arrs["gru_rec_kernel"],
            "gru_bias": arrs["gru_bias"],
            "w1": arrs["w1"], "b1": arrs["b1"],
            "w2": arrs["w2"], "b2": arrs["b2"],
            "w3": np.ascontiguousarray(arrs["w3"][:, VS * c:VS * (c + 1)]),
            "b3": np.ascontiguousarray(arrs["b3"][VS * c:VS * (c + 1)]),
        })
    flags = (bool(np.any(arrs["b3"])), bool(np.any(arrs["gru_bias"])))
    return in_maps, flags


def kernel(**inputs):
    global LAST_RESULT
    in_maps, (has_b3, has_gb) = _make_in_maps(inputs)
    nc = _get_nc(has_b3, has_gb)
    res = run_bass_kernel_spmd(nc, in_maps, core_ids=list(range(NCORES)),
                               trace=TRACE, **TRACE_KWARGS)
    LAST_RESULT = res
    full = np.empty((B, T, V), np.float32)
    for c in range(NCORES):
        full[:, :, VS * c:VS * (c + 1)] = res.results[c]["out"].reshape(B, T, VS)
    return full


# revision 35
# speedup vs baseline: 1.0262x; 1.0262x over previous
"""Trainium2 Bass kernel for nn_Decoder (GRU decoder + MLP + vocab softmax).

Sharding (8 NeuronCores):
  - GRU + 2-layer MLP: data-parallel over batch (4 examples/core).
    Local tokens are b-major (col = b*128 + t) so the global token index
    G = 512*rank + b*128 + t equals example*128 + t, matching output rows.
  - h2^T all-gathered (bf16, 2 chunked collectives) across cores.
  - Final [512,32000] vocab projection + softmax: column-parallel
    (4000 vocab cols/core) with AllReduce'd softmax denominators.

Compute dtypes: bf16 matmul operands, fp32 PSUM accumulation and gates,
bf16 hidden state and exp store (verified ~2.4e-3 rel err vs the fp32
reference; gate is 2e-2).

The softmax skips max-subtraction: logits for this model are O(+-2), far
inside exp's fp32 range, and exp+rowsum are fused in one ScalarE pass via
accum_out.
"""

import numpy as np

import concourse.bass as bass
import concourse.tile as tile
from concourse import bacc, mybir
from concourse.bass import ds, ts
from concourse.bass_utils import run_bass_kernel_spmd
from concourse.masks import make_identity

P = 128
NCORES = 8
B, T, E, H, V = 32, 128, 256, 512, 32000
BL = B // NCORES            # 4 examples per core
NTOK = BL * T               # 512 local tokens
G = B * T                   # 4096 global tokens
VS = V // NCORES            # 4000 vocab cols per core
KO = H // P                 # 4 hidden chunks
MO3 = 3 * H // P            # 12 gate chunks (z:0-3, r:4-7, h:8-11)
SO = (E + H) // P           # 6 input chunks
NJ = 8                      # vocab sub-chunks per token tile (8 x 500)
VC = VS // NJ               # 500
NQ = 2                      # gather chunks
ROUND_SIZES = [6, 6, 6, 6, 6, 2]  # token-tiles per softmax all-reduce round
ROUNDS = len(ROUND_SIZES)
TPR = max(ROUND_SIZES)

f32 = mybir.dt.float32
bf16 = mybir.dt.bfloat16
fp8 = mybir.dt.float8e4

R_FP8 = False

TRACE = False
TRACE_KWARGS = {}
LAST_RESULT = None

RG = [list(range(NCORES))]


def _build(has_b3: bool, has_gb: bool, debug: str | None = None):
    nc = bacc.Bacc("TRN2", target_bir_lowering=False, debug=False,
                   num_devices=NCORES)

    enc_ext = nc.dram_tensor("encoder_input", [BL, T, E], f32, kind="ExternalInput").ap()
    dec_ext = nc.dram_tensor("decoder_input", [BL, H], f32, kind="ExternalInput").ap()
    gk_ext = nc.dram_tensor("gru_kernel", [E + H, 3 * H], f32, kind="ExternalInput").ap()
    gr_ext = nc.dram_tensor("gru_rec_kernel", [H, 3 * H], f32, kind="ExternalInput").ap()
    gb_ext = nc.dram_tensor("gru_bias", [2, 3 * H], f32, kind="ExternalInput").ap()
    w1_ext = nc.dram_tensor("w1", [H, H], f32, kind="ExternalInput").ap()
    b1_ext = nc.dram_tensor("b1", [H], f32, kind="ExternalInput").ap()
    w2_ext = nc.dram_tensor("w2", [H, H], f32, kind="ExternalInput").ap()
    b2_ext = nc.dram_tensor("b2", [H], f32, kind="ExternalInput").ap()
    w3_ext = nc.dram_tensor("w3", [H, VS], f32, kind="ExternalInput").ap()
    b3_ext = nc.dram_tensor("b3", [VS], f32, kind="ExternalInput").ap()

    out_ext = nc.dram_tensor("out", [G, VS], f32, kind="ExternalOutput").ap()
    dbg_ext = None
    if debug == "xproj":
        dbg_ext = nc.dram_tensor("dbg", [P, MO3, NTOK], f32, kind="ExternalOutput").ap()
    elif debug == "hseq":
        dbg_ext = nc.dram_tensor("dbg", [P, KO, NTOK], f32, kind="ExternalOutput").ap()
    elif debug == "h2g":
        dbg_ext = nc.dram_tensor("dbg", [P, KO, G], f32, kind="ExternalOutput").ap()

    with tile.TileContext(nc) as tc:
        with tc.tile_pool(name="dram", bufs=1, space="DRAM") as dram_pool:
            h2_bounce = [dram_pool.tile([H, NTOK // NQ], bf16, name=f"h2b_{q}")
                         for q in range(NQ)]
            h2_gath = [dram_pool.tile([NCORES * H, NTOK // NQ], bf16,
                                      addr_space="Shared",
                                      name=f"h2g_{q}") for q in range(NQ)]
            sums_in = [dram_pool.tile([P * ROUND_SIZES[r]], f32,
                                      name=f"sums_in_{r}")
                       for r in range(ROUNDS)]
            sums_out = [dram_pool.tile([P * ROUND_SIZES[r]], f32,
                                       addr_space="Shared",
                                       name=f"sums_out_{r}")
                        for r in range(ROUNDS)]
            _build_body(nc, tc, has_b3, has_gb, debug, dbg_ext,
                        enc_ext, dec_ext, gk_ext, gr_ext, gb_ext,
                        w1_ext, b1_ext, w2_ext, b2_ext, w3_ext, b3_ext,
                        out_ext, h2_bounce, h2_gath, sums_in, sums_out)
    nc.finalize()
    return nc


def _build_body(nc, tc, has_b3, has_gb, debug, dbg_ext,
                enc_ext, dec_ext, gk_ext, gr_ext, gb_ext,
                w1_ext, b1_ext, w2_ext, b2_ext, w3_ext, b3_ext,
                out_ext, h2_bounce, h2_gath, sums_in, sums_out):
    from contextlib import ExitStack

    Ident = mybir.ActivationFunctionType.Identity
    Sig = mybir.ActivationFunctionType.Sigmoid
    Relu = mybir.ActivationFunctionType.Relu
    Exp = mybir.ActivationFunctionType.Exp

    persist = ExitStack()
    wpool = persist.enter_context(tc.tile_pool(name="wpool", bufs=1))
    w3b = wpool.tile([P, KO, VS], bf16)
    b3bc = wpool.tile([P, VS], f32, name="b3bc") if has_b3 else None

    gru_stack = ExitStack()
    gpool = gru_stack.enter_context(tc.tile_pool(name="gpool", bufs=1))
    w1b = gpool.tile([P, KO, H], bf16)
    w2b = gpool.tile([P, KO, H], bf16)
    b1T = gpool.tile([P, KO], f32)
    b2T = gpool.tile([P, KO], f32)
    h2T = gpool.tile([P, KO, NTOK], bf16)
    gt_pool = gru_stack.enter_context(tc.tile_pool(name="gt", bufs=3))
    psum_pro = gru_stack.enter_context(tc.tile_pool(name="ps_pro", bufs=2, space="PSUM"))
    psum_rec = gru_stack.enter_context(tc.tile_pool(name="ps_rec", bufs=2, space="PSUM"))

    Rdt = fp8 if R_FP8 else bf16
    Rb = gpool.tile([P, KO, 3 * H], Rdt)
    Wkb = gpool.tile([P, SO, 3 * H], bf16)
    seqT = gpool.tile([P, SO, NTOK], bf16)
    xprojT = gpool.tile([P, MO3, NTOK], bf16)
    hseqT = gpool.tile([P, KO, NTOK], bf16)
    h1T = gpool.tile([P, KO, NTOK], bf16)

    # ---- input loads: fp32 DRAM -> SBUF, chunked, cast on DVE ----
    wtmp_pool = gru_stack.enter_context(tc.tile_pool(name="wtmp", bufs=3))

    def load_cast(dst3, src_ext, nck, width, tag):
        src_c = src_ext.rearrange("(k p) m -> k p m", p=P)
        for k in range(nck):
            tmp = wtmp_pool.tile([P, width], f32, tag=tag, name=f"{tag}_{k}")
            nc.sync.dma_start(out=tmp[:], in_=src_c[k])
            nc.vector.tensor_copy(out=dst3[:, k], in_=tmp[:])

    # encoder: natural load (contiguous rows), cast, PE-transpose into seqT
    seqT4 = seqT.rearrange("p so (b t) -> p so b t", b=BL)
    ident = gpool.tile([P, P], bf16)
    make_identity(nc, ident)
    enc_nat = gpool.tile([P, BL, E], f32)
    nc.sync.dma_start(out=enc_nat[:], in_=enc_ext.rearrange("b t c -> t b c"))
    enc_natb = gpool.tile([P, BL, E], bf16)
    nc.vector.tensor_copy(out=enc_natb[:], in_=enc_nat[:])
    for b in range(BL):
        for co in range(E // P):
            pst = psum_pro.tile([P, P], bf16, tag="pro", name=f"tp_{b}_{co}")
            nc.tensor.transpose(pst[:], enc_natb[:, b, ts(co, P)], ident)
            nc.vector.tensor_copy(out=seqT4[:, co, b, :], in_=pst[:])
    # decoder -> bf16, broadcast over t into seqT chunks 2-5
    decT = gpool.tile([P, KO, BL], f32)
    for b in range(BL):
        nc.sync.dma_start(out=decT[:, :, b],
                          in_=dec_ext[b].rearrange("(ko p) -> p ko", p=P))
    decTb = gpool.tile([P, KO, BL], bf16)
    nc.vector.tensor_copy(out=decTb[:], in_=decT[:])
    nc.vector.tensor_copy(out=seqT4[:, E // P:SO],
                          in_=decTb[:, :, :, None].to_broadcast((P, KO, BL, T)))

    load_cast(Wkb, gk_ext, SO, 3 * H, "wtmp")
    load_cast(Rb, gr_ext, KO, 3 * H, "wtmp")
    if has_b3:
        b3_brd = bass.AP(tensor=b3_ext.tensor, offset=b3_ext.offset,
                         ap=[[0, P]] + list(b3_ext.ap))
        nc.sync.dma_start(out=b3bc[:], in_=b3_brd)

    # gru biases (generic path; skipped when zero)
    if has_gb:
        gbT = gpool.tile([P, MO3, 2], f32)
        for i in range(2):
            nc.sync.dma_start(out=gbT[:, :, i],
                              in_=gb_ext[i].rearrange("(mo p) -> p mo", p=P))
        xbias = gpool.tile([P, MO3], f32)
        nc.vector.tensor_copy(out=xbias[:], in_=gbT[:, :, 0])
        nc.vector.tensor_add(out=xbias[:, 0:8], in0=xbias[:, 0:8], in1=gbT[:, 0:8, 1])
        brecH = gpool.tile([P, KO, BL], f32)
        nc.vector.tensor_copy(out=brecH[:],
                              in_=gbT[:, 8:12, 1:2].to_broadcast((P, KO, BL)))

    # ---- x_proj^T = Wk^T @ seq^T (+bias), chunked by timestep range ----
    # chunk 0 is emitted here (gates the first GRU steps); chunks 1..3 are
    # emitted after the GRU loop so the scheduler back-fills them into the
    # GRU's PE gaps instead of delaying the scan start.
    XC = 4
    XS = T // XC  # 32 steps per chunk
    seq_bt = seqT.rearrange("p so (b t) -> p so b t", b=BL)
    xp_bt = xprojT.rearrange("p m (b t) -> p m b t", b=BL)

    def emit_xproj_chunk(c):
        for m in range(MO3):
            ps = psum_pro.tile([P, BL * XS], f32, tag="pro", name=f"xp_{c}_{m}")
            for k in range(SO):
                nc.tensor.matmul(ps[:], lhsT=Wkb[:, k, ts(m, P)],
                                 rhs=seq_bt[:, k, :, ds(XS * c, XS)],
                                 start=(k == 0), stop=(k == SO - 1))
            dst = xp_bt[:, m, :, ds(XS * c, XS)]
            if has_gb:
                nc.scalar.activation(out=dst, in_=ps[:], func=Ident,
                                     bias=xbias[:, m:m + 1])
            else:
                nc.scalar.copy(out=dst, in_=ps[:])

    emit_xproj_chunk(0)

    if debug == "xproj":
        nc.sync.dma_start(out=dbg_ext, in_=xprojT[:])

    # ---- GRU scan (t-major local tokens) ----
    xp4 = xprojT.rearrange("p m (b t) -> p m b t", b=BL)
    hs4 = hseqT.rearrange("p ko (b t) -> p ko b t", b=BL)

    # t = 0 (h == 0): z,r = sig(xz), hh = relu(xh [+ r*brecH]), h = (1-z)*hh
    zr0 = gt_pool.tile([P, 8, BL], f32, tag="zr")
    nc.scalar.activation(out=zr0[:], in_=xp4[:, 0:8, :, 0], func=Sig)
    hh0 = gt_pool.tile([P, KO, BL], f32, tag="hh")
    if has_gb:
        nc.vector.tensor_mul(out=hh0[:], in0=zr0[:, 4:8], in1=brecH[:])
        nc.vector.tensor_add(out=hh0[:], in0=hh0[:], in1=xp4[:, 8:12, :, 0])
        nc.vector.tensor_scalar_max(hh0[:], hh0[:], 0.0)
    else:
        nc.vector.tensor_scalar_max(hh0[:], xp4[:, 8:12, :, 0], 0.0)
    d0 = gt_pool.tile([P, KO, BL], f32, tag="d")
    nc.vector.tensor_mul(out=d0[:], in0=zr0[:, 0:4], in1=hh0[:])
    nc.vector.tensor_sub(out=hs4[:, :, :, 0], in0=hh0[:], in1=d0[:])

    for t in range(1, T):
        if t % XS == XS - 8 and t // XS + 1 < XC:
            emit_xproj_chunk(t // XS + 1)
        r_ps = psum_rec.tile([P, KO * BL], f32, tag="r_ps", name=f"rp_{t}")
        h_ps = psum_rec.tile([P, KO * BL], f32, tag="h_ps", name=f"hp_{t}")
        z_ps = psum_rec.tile([P, KO * BL], f32, tag="z_ps", name=f"zp_{t}")
        nc.tensor.matmul(r_ps[:], lhsT=ident, rhs=xp4[:, 4:8, :, t],
                         start=True, stop=False)
        for m in range(4):
            for ko in range(KO):
                nc.tensor.matmul(r_ps[:, ds(BL * m, BL)],
                                 lhsT=Rb[:, ko, ts(4 + m, P)],
                                 rhs=hs4[:, ko, :, t - 1],
                                 start=False, stop=(ko == KO - 1) and (m == 3))
        for m in range(4):
            for ko in range(KO):
                nc.tensor.matmul(h_ps[:, ds(BL * m, BL)],
                                 lhsT=Rb[:, ko, ts(8 + m, P)],
                                 rhs=hs4[:, ko, :, t - 1],
                                 start=(ko == 0), stop=(ko == KO - 1))
        nc.tensor.matmul(z_ps[:], lhsT=ident, rhs=xp4[:, 0:4, :, t],
                         start=True, stop=False)
        for m in range(4):
            for ko in range(KO):
                nc.tensor.matmul(z_ps[:, ds(BL * m, BL)],
                                 lhsT=Rb[:, ko, ts(m, P)],
                                 rhs=hs4[:, ko, :, t - 1],
                                 start=False, stop=(ko == KO - 1) and (m == 3))
        rr = gt_pool.tile([P, KO, BL], f32, tag="rr", name=f"rr_{t}")
        nc.scalar.activation(out=rr[:],
                             in_=r_ps.rearrange("p (m b) -> p m b", b=BL), func=Sig)
        hh = gt_pool.tile([P, KO, BL], f32, tag="hh", name=f"hh_{t}")
        hp4 = h_ps.rearrange("p (m b) -> p m b", b=BL)
        if has_gb:
            nc.vector.tensor_add(out=hh[:], in0=hp4, in1=brecH[:])
            nc.vector.tensor_mul(out=hh[:], in0=rr[:], in1=hh[:])
        else:
            nc.vector.tensor_mul(out=hh[:], in0=rr[:], in1=hp4)
        nc.vector.tensor_add(out=hh[:], in0=hh[:], in1=xp4[:, 8:12, :, t])
        nc.vector.tensor_scalar_max(hh[:], hh[:], 0.0)
        dd = gt_pool.tile([P, KO, BL], f32, tag="d", name=f"d_{t}")
        nc.vector.tensor_sub(out=dd[:], in0=hs4[:, :, :, t - 1], in1=hh[:])
        zz = gt_pool.tile([P, KO, BL], f32, tag="zz", name=f"zz_{t}")
        nc.scalar.activation(out=zz[:],
                             in_=z_ps.rearrange("p (m b) -> p m b", b=BL), func=Sig)
        nc.vector.tensor_mul(out=dd[:], in0=zz[:], in1=dd[:])
        nc.vector.tensor_add(out=hs4[:, :, :, t], in0=hh[:], in1=dd[:])

    if debug == "hseq":
        dbgf = gpool.tile([P, KO, NTOK], f32)
        nc.vector.tensor_copy(out=dbgf[:], in_=hseqT[:])
        nc.sync.dma_start(out=dbg_ext, in_=dbgf[:])

    # deferred weight loads (DMAs overlap the GRU; w3 casts happen during
    # the gather window so they don't interrupt GRU gate chains)
    load_cast(w1b, w1_ext, KO, H, "wtmp")
    load_cast(w2b, w2_ext, KO, H, "wtmp")
    nc.sync.dma_start(out=b1T[:], in_=b1_ext.rearrange("(mo p) -> p mo", p=P))
    nc.sync.dma_start(out=b2T[:], in_=b2_ext.rearrange("(mo p) -> p mo", p=P))
    w3_c = w3_ext.rearrange("(k p) m -> k p m", p=P)
    for k in range(KO):
        tmp = wtmp_pool.tile([P, VS], f32, tag="w3tmp", name=f"w3tmp_{k}")
        nc.sync.dma_start(out=tmp[:], in_=w3_c[k])
        nc.vector.tensor_copy(out=w3b[:, k, 0:VS // 2], in_=tmp[:, 0:VS // 2])
        nc.vector.tensor_copy(out=w3b[:, k, VS // 2:], in_=tmp[:, VS // 2:])

    # ---- MLP (both layers chunked per gather half) ----
    HT = NTOK // NQ
    for q in range(NQ):
        for m in range(KO):
            ps = psum_pro.tile([P, HT], f32, tag="pro", name=f"m1_{q}_{m}")
            for k in range(KO):
                nc.tensor.matmul(ps[:], lhsT=w1b[:, k, ts(m, P)],
                                 rhs=hseqT[:, k, ds(HT * q, HT)],
                                 start=(k == 0), stop=(k == KO - 1))
            nc.scalar.activation(out=h1T[:, m, ds(HT * q, HT)], in_=ps[:],
                                 func=Relu, bias=b1T[:, m:m + 1])
        for m in range(KO):
            ps = psum_pro.tile([P, HT], f32, tag="pro", name=f"m2_{q}_{m}")
            for k in range(KO):
                nc.tensor.matmul(ps[:], lhsT=w2b[:, k, ts(m, P)],
                                 rhs=h1T[:, k, ds(HT * q, HT)],
                                 start=(k == 0), stop=(k == KO - 1))
            nc.scalar.activation(out=h2T[:, m, ds(HT * q, HT)], in_=ps[:],
                                 func=Relu, bias=b2T[:, m:m + 1])
        nc.gpsimd.dma_start(out=h2_bounce[q].rearrange("(ko p) t -> p ko t", p=P),
                            in_=h2T[:, :, ds(HT * q, HT)])
        nc.gpsimd.collective_compute(
            "AllGather", mybir.AluOpType.bypass,
            ins=[h2_bounce[q].opt()], outs=[h2_gath[q].opt()],
            replica_groups=RG,
        )

    gru_stack.close()

    voc_stack = ExitStack()
    vpool = voc_stack.enter_context(tc.tile_pool(name="vpool", bufs=1))
    exp_pool = voc_stack.enter_context(tc.tile_pool(name="exp", bufs=12))
    out_pool = voc_stack.enter_context(tc.tile_pool(name="outp", bufs=2))
    sc_pool = voc_stack.enter_context(tc.tile_pool(name="scp", bufs=3))
    psum_voc = voc_stack.enter_context(tc.tile_pool(name="ps_voc", bufs=2, space="PSUM"))

    h2gT = vpool.tile([P, KO, G], bf16)
    h2g_q = h2gT.rearrange("p ko (r q t) -> p ko r q t", r=NCORES, q=NQ)
    for q in range(NQ):
        src = h2_gath[q].rearrange("(r ko p) t -> ko p r t", p=P, ko=KO)
        for ko in range(KO):
            nc.scalar.dma_start(out=h2g_q[:, ko, :, q, :], in_=src[ko])

    if debug == "h2g":
        dbgf = vpool.tile([P, KO, G], f32)
        nc.vector.tensor_copy(out=dbgf[:], in_=h2gT[:])
        nc.sync.dma_start(out=dbg_ext, in_=dbgf[:])

    # vocab tiles ordered quarter-major so the first tiles only need AG q=0
    tile_order = [h * (2 * NCORES) + i for h in range(NQ)
                  for i in range(2 * NCORES)]
    # tile gt covers G rows [128*gt, 128*gt+128): rank gt//4, gather half
    # (gt%4)//2 since each rank contributes 512 tokens = 4 tiles = 2 halves.
    proc = []
    it = iter(tile_order)
    for sz in ROUND_SIZES:
        proc.append([next(it) for _ in range(sz)])

    pending = None  # (exps, proc_list, rcp) of the previous round, scaled late

    def emit_scales(pend):
        exps_p, proc_p, rcp_p, rnd_p = pend
        for i, gt in enumerate(proc_p):
            ob = out_pool.tile([P, NJ, VC], f32, tag="ob", name=f"ob_{gt}")
            if rnd_p >= ROUNDS - 2 and i % 2:
                nc.scalar.activation(out=ob[:], in_=exps_p[i][:],
                                     func=mybir.ActivationFunctionType.Copy,
                                     scale=rcp_p[:, i:i + 1])
            else:
                nc.vector.tensor_scalar_mul(ob[:], exps_p[i][:], rcp_p[:, i:i + 1])
            nc.sync.dma_start(out=out_ext[ds(P * gt, P), :],
                              in_=ob.rearrange("p j v -> p (j v)"))

    for rnd in range(ROUNDS):
        nr = ROUND_SIZES[rnd]
        sums = sc_pool.tile([P, TPR, 2], f32, tag="sums", name=f"sums_{rnd}")
        exps = []
        for i, gt in enumerate(proc[rnd]):
            expb = exp_pool.tile([P, NJ, VC], bf16, tag="expb", name=f"expb_{gt}")
            for half in range(2):
                pv = psum_voc.tile([P, NJ // 2, 512], f32, tag="pv",
                                   name=f"pv_{gt}_{half}")
                for ko in range(KO):
                    last = (ko == KO - 1) and not has_b3
                    for j in range(NJ // 2):
                        jj = half * (NJ // 2) + j
                        nc.tensor.matmul(pv[:, j, 0:VC],
                                         lhsT=h2gT[:, ko, ts(gt, P)],
                                         rhs=w3b[:, ko, ds(VC * jj, VC)],
                                         start=(ko == 0), stop=last)
                if has_b3:
                    b3v = b3bc[:, ds(VC * half * (NJ // 2), VC * (NJ // 2))]
                    nc.vector.tensor_add(
                        out=pv[:, :, 0:VC], in0=pv[:, :, 0:VC],
                        in1=b3v.rearrange("p (j v) -> p j v", j=NJ // 2))
                nc.scalar.activation(
                    out=expb[:, ds(half * (NJ // 2), NJ // 2), :],
                    in_=pv[:, :, 0:VC], func=Exp,
                    accum_out=sums[:, i, half:half + 1])
            exps.append(expb)
        ssum = sc_pool.tile([P, TPR], f32, tag="ssum", name=f"ssum_{rnd}")
        nc.vector.tensor_add(out=ssum[:, :nr], in0=sums[:, :nr, 0],
                             in1=sums[:, :nr, 1])
        nc.gpsimd.dma_start(out=sums_in[rnd].rearrange("(i p) -> p i", p=P),
                            in_=ssum[:, :nr])
        nc.gpsimd.collective_compute(
            "AllReduce", mybir.AluOpType.add,
            ins=[sums_in[rnd].opt()], outs=[sums_out[rnd].opt()],
            replica_groups=RG,
        )
        if pending is not None:
            emit_scales(pending)
        rcp = sc_pool.tile([P, TPR], f32, tag="rcp", name=f"rcp_{rnd}")
        nc.scalar.dma_start(out=rcp[:, :nr],
                            in_=sums_out[rnd].rearrange("(i p) -> p i", p=P))
        nc.vector.reciprocal(out=rcp[:, :nr], in_=rcp[:, :nr])
        pending = (exps, proc[rnd], rcp, rnd)

    emit_scales(pending)

    voc_stack.close()
    persist.close()


_BUILD_CACHE = {}


def _get_nc(has_b3: bool, has_gb: bool, debug=None):
    key = (has_b3, has_gb, debug)
    if key not in _BUILD_CACHE:
        _BUILD_CACHE[key] = _build(has_b3, has_gb, debug)
    return _BUILD_CACHE[key]


def _make_in_maps(inputs):
    arrs = {k: np.ascontiguousarray(np.asarray(v, dtype=np.float32))
            for k, v in inputs.items()}
    in_maps = []
    for c in range(NCORES):
        in_maps.append({
            "encoder_input": arrs["encoder_input"][BL * c:BL * (c + 1)],
            "decoder_input": arrs["decoder_input"][BL * c:BL * (c + 1)],
            "gru_kernel": arrs["gru_kernel"],
            "gru_rec_kernel": arrs["gru_rec_kernel"],
            "gru_bias": arrs["gru_bias"],
            "w1": arrs["w1"], "b1": arrs["b1"],
            "w2": arrs["w2"], "b2": arrs["b2"],
            "w3": np.ascontiguousarray(arrs["w3"][:, VS * c:VS * (c + 1)]),
            "b3": np.ascontiguousarray(arrs["b3"][VS * c:VS * (c + 1)]),
        })
    flags = (bool(np.any(arrs["b3"])), bool(np.any(arrs["gru_bias"])))
    return in_maps, flags


def kernel(**inputs):
    global LAST_RESULT
    in_maps, (has_b3, has_gb) = _make_in_maps(inputs)
    nc = _get_nc(has_b3, has_gb)
    res = run_bass_kernel_spmd(nc, in_maps, core_ids=list(range(NCORES)),
                               trace=TRACE, **TRACE_KWARGS)
    LAST_RESULT = res
    full = np.empty((B, T, V), np.float32)
    for c in range(NCORES):
        full[:, :, VS * c:VS * (c + 1)] = res.results[c]["out"].reshape(B, T, VS)
    return full
